# revision 3
# baseline (speedup 1.0000x reference)
"""Trainium2 kernel for nn_DirectForce (gnn_message_passing).

Math (see reference):
    h   = softplus(X @ w1 + b1) - log(2)          per-edge MLP        [E, 64]
    mag = h @ w2 + b2                                                  [E, 1]
    mag = mag - mean_over_center(mag)[center]      scatter-mean debias
    pair-average mag between each directed edge and its reverse edge
    F   = segment_sum(mag * unit_vec, center)                          [N, 3]

The sorted-pair averaging pairs each directed edge with its reverse (same
length, negated vector), so the pair-averaged scatter reduces algebraically to
    F = segsum(0.5*mag*unit, center) - segsum(0.5*mag*unit, neigh)
which removes the argsort entirely.

Device design (8 NeuronCores, SPMD, edges partitioned contiguously 200k/core):
  - features pre-transposed + converted to bf16 on host (halves HBM traffic
    vs f32; final rel err ~4e-3, well under the 2e-2 gate), pre-tiled to
    [NTILES, 128, XT_TILE] so every input DMA is contiguous; tiny MLP
    weights replicated (per the sharding hint).
  - edges processed in superchunks (SC) of 1024 edges = stacked z [128,512]
    (rows 0-63 = chunk-A hidden, 64-127 = chunk-B hidden via zero-padded
    [w1;0]/[0;w1] stationary weights), grouped 3 SC per PSUM z-tile
    [128,1536] (3 banks) so LDWEIGHTS for w1a/w1b amortize over 3 matmuls.
  - softplus = ACT Exp(z+b1) then Ln(e+1), each ONE 1536-wide op per group
    (ACT cost is free-length x 0.83ns + ~175ns/op, so wide ops amortize);
    the activation-table patch pins Exp+Ln to the single table set holding
    both, so the table loads exactly once.  (A 1-pass Softplus ISA op
    exists but its PWP table slot is mis-indexed on this stack - produces
    garbage; probed.)  h is written as bf16.
  - mm2: per SC one matmul [2,512] = w2-blockdiag^T @ h, written into ONE
    shared PSUM bank at partition offsets 0/32/64/96 via tile_position, so
    a single DVE copy per 4 SCs evacuates mag (38us DVE total vs 137us if
    copied per-SC), converting to bf16; DMA out on the Activation HWDGE
    queue (input stream owns the Sync queue).
  - PE program order per group g: mm1(g) x6 then mm2(g-1) x3, so the PE
    works on mm1(g) while ACT processes group g-1; ACT (the bottleneck at
    ~190us/core) never waits.
Host post (index-structured tail, numpy): debias via bincount, unit
vectors, two segment sums.
"""

import numpy as np
import ml_dtypes

N_ATOMS = 50000
E_TOT = 1600000
D_FEAT = 128
H_DIM = 64
N_CORES = 8
EC = E_TOT // N_CORES          # 200000 edges per core
SC = 1024                      # edges per superchunk (2 chunks of 512)
GRP = 3                        # superchunks per z-tile / ACT op
NSC = 198                      # padded superchunk count (multiple of GRP)
ECP = NSC * SC                 # 202752 padded edges per core
NGROUPS = NSC // GRP           # 66
NBANK = (NSC + 3) // 4         # 50 mag PSUM banks (4 SC each)

# input-tile taper in superchunks (each a multiple of GRP): small head tiles
# so compute starts fast, 12-SC (3 MiB bf16) tiles in the middle
TILE_SIZES = [3, 3, 6] + [12] * 15 + [6]
assert sum(TILE_SIZES) == NSC
NTILES = len(TILE_SIZES)
XT_TILE = max(TILE_SIZES) * SC  # 12288

_CACHE = {}
LAST_RESULTS = None


def _patch_act_tables():
    """Pin Exp and Ln to the one table set containing both
    (natural_log_exp_and_others) so the ACT table loads exactly once.
    Table-set ids are positional, so keys/order are preserved."""
    import functools
    import concourse.hw_specs as hw_specs
    import concourse.bacc as bacc_mod
    import concourse.mybir as mybir

    if _CACHE.get("tables_patched"):
        return
    orig = hw_specs.get_activation_tables
    Exp = mybir.ActivationFunctionType.Exp
    Ln = mybir.ActivationFunctionType.Ln

    def patched(arch):
        out = {}
        for name, fns in orig(arch).items():
            if name != "natural_log_exp_and_others":
                fns = fns - {Exp, Ln}
            out[name] = fns
        return out

    cached = functools.cache(patched)
    hw_specs.get_activation_tables = cached
    bacc_mod.get_activation_tables = cached
    _CACHE["tables_patched"] = True


def _build_nc():
    import concourse.bacc as bacc
    import concourse.mybir as mybir
    import concourse.tile as tile

    _patch_act_tables()

    F32 = mybir.dt.float32
    BF16 = mybir.dt.bfloat16
    Exp = mybir.ActivationFunctionType.Exp
    Ln = mybir.ActivationFunctionType.Ln

    nc = bacc.Bacc("TRN2", target_bir_lowering=False, debug=False)
    xt_d = nc.dram_tensor("xt", [NTILES, 128, XT_TILE], BF16, kind="ExternalInput")
    w1a_d = nc.dram_tensor("w1a", [128, 128], BF16, kind="ExternalInput")
    w1b_d = nc.dram_tensor("w1b", [128, 128], BF16, kind="ExternalInput")
    b1_d = nc.dram_tensor("b1s", [128, 1], F32, kind="ExternalInput")
    w2_d = nc.dram_tensor("w2d", [128, 2], BF16, kind="ExternalInput")
    mag_d = nc.dram_tensor("mag", [NBANK, 128, 512], BF16, kind="ExternalOutput")

    with tile.TileContext(nc) as tc:
        with (
            tc.tile_pool(name="wp", bufs=1) as wp,
            tc.tile_pool(name="xp", bufs=3) as xp,
            tc.tile_pool(name="ep", bufs=2) as ep,
            tc.tile_pool(name="hp", bufs=3) as hp,
            tc.tile_pool(name="mp", bufs=3) as mp,
            tc.tile_pool(name="zp", bufs=2, space="PSUM") as zp,
            tc.tile_pool(name="magp", bufs=2, space="PSUM") as magp,
        ):
            w1a = wp.tile([128, 128], BF16, tag="w1a")
            w1b = wp.tile([128, 128], BF16, tag="w1b")
            b1s = wp.tile([128, 1], F32, tag="b1s")
            w2d = wp.tile([128, 2], BF16, tag="w2d")
            nc.gpsimd.dma_start(w1a[:], w1a_d[:])
            nc.gpsimd.dma_start(w1b[:], w1b_d[:])
            nc.gpsimd.dma_start(b1s[:], b1_d[:])
            nc.gpsimd.dma_start(w2d[:], w2_d[:])

            # per-group pending mm2 work: (h_tile, list of (slice, sc))
            pending = None
            mag_t = None
            mag_sb = None

            def emit_mm2(pend):
                nonlocal mag_t, mag_sb
                h_t, scs = pend
                for si, sc_abs in enumerate(scs):
                    slot = sc_abs % 4
                    if slot == 0:
                        mag_t = magp.tile([128, 512], F32, tag="magt")
                    off = 32 * slot
                    nc.tensor.matmul(
                        mag_t[off:off + 2, :], w2d[:],
                        h_t[:, si * 512:(si + 1) * 512],
                        start=True, stop=True, tile_position=(0, off),
                    )
                    if slot == 3 or sc_abs == NSC - 1:
                        bank = sc_abs // 4
                        mag_sb = mp.tile([128, 512], BF16, tag="magsb")
                        nc.vector.tensor_copy(mag_sb[:], mag_t[:])
                        nc.scalar.dma_start(mag_d[bank], mag_sb[:])

            sc_abs = 0
            for ti, size in enumerate(TILE_SIZES):
                width = size * SC
                xt = xp.tile([128, XT_TILE], BF16, tag="xt")
                nc.sync.dma_start(xt[:, :width], xt_d[ti, :, :width])
                for g in range(size // GRP):
                    base = g * GRP * SC
                    z = zp.tile([128, GRP * 512], F32, tag="z")
                    # mm1: A-halves with w1a stationary, then B-halves w1b
                    for s in range(GRP):
                        nc.tensor.matmul(
                            z[:, s * 512:(s + 1) * 512], w1a[:],
                            xt[:, base + s * SC:base + s * SC + 512],
                            start=True, stop=False,
                        )
                    for s in range(GRP):
                        nc.tensor.matmul(
                            z[:, s * 512:(s + 1) * 512], w1b[:],
                            xt[:, base + s * SC + 512:base + s * SC + 1024],
                            start=False, stop=True,
                        )
                    # pipelined mm2 of the previous group (h ready by now)
                    if pending is not None:
                        emit_mm2(pending)
                    # softplus: Exp then Ln, one wide op each
                    e_t = ep.tile([128, GRP * 512], F32, tag="e")
                    nc.scalar.activation(e_t[:], z[:], Exp, bias=b1s[:, :1])
                    h_t = hp.tile([128, GRP * 512], BF16, tag="h")
                    nc.scalar.activation(h_t[:], e_t[:], Ln, bias=1.0)
                    pending = (h_t, [sc_abs + s for s in range(GRP)])
                    sc_abs += GRP
            emit_mm2(pending)
    nc.compile()
    return nc


def _get_nc():
    if "nc" not in _CACHE:
        _CACHE["nc"] = _build_nc()
    return _CACHE["nc"]


def kernel(features, edge_vectors, edge_lengths, edge_index, w1, b1, w2, b2):
    global LAST_RESULTS
    from concourse.bass_utils import run_bass_kernel_spmd

    BF = ml_dtypes.bfloat16
    features = np.asarray(features, dtype=np.float32)
    edge_vectors = np.asarray(edge_vectors, dtype=np.float32)
    edge_lengths = np.asarray(edge_lengths, dtype=np.float32)
    edge_index = np.asarray(edge_index)
    w1 = np.asarray(w1, dtype=np.float32)
    b1 = np.asarray(b1, dtype=np.float32).reshape(-1)
    w2 = np.asarray(w2, dtype=np.float32).reshape(-1, 1)
    b2 = np.asarray(b2, dtype=np.float32).reshape(-1)

    # replicated small weights, padded for the stacked-z / block-diag tricks
    w1a = np.zeros((128, 128), np.float32)
    w1a[:, :H_DIM] = w1
    w1b = np.zeros((128, 128), np.float32)
    w1b[:, H_DIM:] = w1
    b1s = np.concatenate([b1, b1]).astype(np.float32).reshape(128, 1)
    w2d = np.zeros((128, 2), np.float32)
    w2d[:H_DIM, 0] = w2[:, 0]
    w2d[H_DIM:, 1] = w2[:, 0]

    feats_bf = features.astype(BF)

    # shard edges contiguously across cores; per-core transposed bf16 panel
    in_maps = []
    for c in range(N_CORES):
        sl = slice(c * EC, (c + 1) * EC)
        panel = np.zeros((128, ECP), BF)
        panel[:, :EC] = feats_bf[sl].T
        xt = np.zeros((NTILES, 128, XT_TILE), BF)
        a = 0
        for ti, size in enumerate(TILE_SIZES):
            w = size * SC
            xt[ti, :, :w] = panel[:, a:a + w]
            a += w
        in_maps.append({
            "xt": xt,
            "w1a": w1a.astype(BF), "w1b": w1b.astype(BF),
            "b1s": b1s, "w2d": w2d.astype(BF),
        })

    nc = _get_nc()
    try:
        res = run_bass_kernel_spmd(nc, in_maps, core_ids=list(range(N_CORES)))
    except Exception:
        # one retry for transient runtime failures
        import time
        time.sleep(2.0)
        res = run_bass_kernel_spmd(nc, in_maps, core_ids=list(range(N_CORES)))
    LAST_RESULTS = res

    # decode mag: [NBANK, 128, 512] bf16; sc = 4*bank + slot, partition rows
    # 32*slot (+0 = A-edges col c -> edge sc*1024+c, +1 = B-edges +512)
    mag = np.empty(E_TOT, np.float32)
    for c in range(N_CORES):
        arr = np.asarray(res.results[c]["mag"], dtype=BF).astype(np.float32)
        mr = arr.reshape(NBANK, 4, 32, 512)[:, :, :2, :].reshape(NBANK * 4, 2, 512)
        mag[c * EC:(c + 1) * EC] = mr[:NSC].reshape(-1)[:EC]

    # fold b2 and the shifted-softplus constant: h_ref = h_dev - log(2)
    mag = mag + (b2[0] - np.float32(np.log(2.0)) * w2.sum())

    center = edge_index[0].astype(np.int64)
    neigh = edge_index[1].astype(np.int64)

    # scatter-mean debias per center atom
    cnt = np.bincount(center, minlength=N_ATOMS).astype(np.float32)
    ssum = np.bincount(center, weights=mag.astype(np.float64), minlength=N_ATOMS)
    bias = (ssum / np.maximum(cnt, 1.0)).astype(np.float32)
    mag = mag - bias[center]

    # pair-averaged antisymmetric force assembly (see module docstring)
    unit = edge_vectors / edge_lengths[:, None]
    val = (0.5 * mag)[:, None] * unit  # [E, 3]
    forces = np.zeros((N_ATOMS, 3), np.float32)
    for k in range(3):
        fc = np.bincount(center, weights=val[:, k].astype(np.float64), minlength=N_ATOMS)
        fn = np.bincount(neigh, weights=val[:, k].astype(np.float64), minlength=N_ATOMS)
        forces[:, k] = (fc - fn).astype(np.float32)
    return forces


# revision 4
# speedup vs baseline: 1.1349x; 1.1349x over previous
"""Trainium2 kernel for nn_DirectForce (gnn_message_passing).

Math (see reference):
    h   = softplus(X @ w1 + b1) - log(2)          per-edge MLP        [E, 64]
    mag = h @ w2 + b2                                                  [E, 1]
    mag = mag - mean_over_center(mag)[center]      scatter-mean debias
    pair-average mag between each directed edge and its reverse edge
    F   = segment_sum(mag * unit_vec, center)                          [N, 3]

The sorted-pair averaging pairs each directed edge with its reverse (same
length, negated vector), so the pair-averaged scatter reduces algebraically to
    F = segsum(0.5*mag*unit, center) - segsum(0.5*mag*unit, neigh)
which removes the argsort entirely.

Device design (8 NeuronCores, SPMD, edges partitioned contiguously 200k/core):
  - features pre-transposed + converted to bf16 on host (halves HBM traffic
    vs f32; final rel err ~4e-3, well under the 2e-2 gate), pre-tiled to
    [NTILES, 128, XT_TILE] so every input DMA is contiguous; tiny MLP
    weights replicated (per the sharding hint).
  - edges processed in superchunks (SC) of 1024 edges = stacked z [128,512]
    (rows 0-63 = chunk-A hidden, 64-127 = chunk-B hidden via zero-padded
    [w1;0]/[0;w1] stationary weights), grouped 3 SC per PSUM z-tile
    [128,1536] (3 banks) so LDWEIGHTS for w1a/w1b amortize over 3 matmuls.
  - softplus = ACT Exp(z+b1) then Ln(e+1), each ONE 1536-wide op per group
    (ACT cost is free-length x 0.83ns + ~175ns/op, so wide ops amortize);
    the activation-table patch pins Exp+Ln to the single table set holding
    both, so the table loads exactly once.  (A 1-pass Softplus ISA op
    exists but its PWP table slot is mis-indexed on this stack - produces
    garbage; probed.)  h is written as bf16.
  - mm2: per SC one matmul [2,512] = w2-blockdiag^T @ h, written into ONE
    shared PSUM bank at partition offsets 0/32/64/96 via tile_position, so
    a single DVE copy per 4 SCs evacuates mag (38us DVE total vs 137us if
    copied per-SC), converting to bf16; DMA out on the Activation HWDGE
    queue (input stream owns the Sync queue).
  - PE program order per group g: mm1(g) x6 then mm2(g-1) x3, so the PE
    works on mm1(g) while ACT processes group g-1; ACT (the bottleneck at
    ~190us/core) never waits.
Host post (index-structured tail, numpy): debias via bincount, unit
vectors, two segment sums.
"""

import numpy as np
import ml_dtypes

N_ATOMS = 50000
E_TOT = 1600000
D_FEAT = 128
H_DIM = 64
N_CORES = 8
EC = E_TOT // N_CORES          # 200000 edges per core
SC = 1024                      # edges per superchunk (2 chunks of 512)
GRP = 3                        # superchunks per z-tile / ACT op
NSC = 198                      # padded superchunk count (multiple of GRP)
ECP = NSC * SC                 # 202752 padded edges per core
NGROUPS = NSC // GRP           # 66
NBANK = (NSC + 3) // 4         # 50 mag PSUM banks (4 SC each)

# input-tile taper in superchunks (each a multiple of GRP): small head tiles
# so compute starts fast, 12-SC (3 MiB bf16) tiles in the middle
TILE_SIZES = [3, 3, 6] + [12] * 15 + [6]
assert sum(TILE_SIZES) == NSC
NTILES = len(TILE_SIZES)
XT_TILE = max(TILE_SIZES) * SC  # 12288

_CACHE = {}
LAST_RESULTS = None


def _patch_act_tables():
    """Pin Exp and Ln to the one table set containing both
    (natural_log_exp_and_others) so the ACT table loads exactly once.
    Table-set ids are positional, so keys/order are preserved."""
    import functools
    import concourse.hw_specs as hw_specs
    import concourse.bacc as bacc_mod
    import concourse.mybir as mybir

    if _CACHE.get("tables_patched"):
        return
    orig = hw_specs.get_activation_tables
    Exp = mybir.ActivationFunctionType.Exp
    Ln = mybir.ActivationFunctionType.Ln

    def patched(arch):
        out = {}
        for name, fns in orig(arch).items():
            if name != "natural_log_exp_and_others":
                fns = fns - {Exp, Ln}
            out[name] = fns
        return out

    cached = functools.cache(patched)
    hw_specs.get_activation_tables = cached
    bacc_mod.get_activation_tables = cached
    _CACHE["tables_patched"] = True


def _build_nc():
    import concourse.bacc as bacc
    import concourse.mybir as mybir
    import concourse.tile as tile

    _patch_act_tables()

    F32 = mybir.dt.float32
    BF16 = mybir.dt.bfloat16
    Exp = mybir.ActivationFunctionType.Exp
    Ln = mybir.ActivationFunctionType.Ln

    nc = bacc.Bacc("TRN2", target_bir_lowering=False, debug=False)
    xt_d = nc.dram_tensor("xt", [NTILES, 128, XT_TILE], BF16, kind="ExternalInput")
    w1a_d = nc.dram_tensor("w1a", [128, 128], BF16, kind="ExternalInput")
    w1b_d = nc.dram_tensor("w1b", [128, 128], BF16, kind="ExternalInput")
    b1_d = nc.dram_tensor("b1s", [128, 1], F32, kind="ExternalInput")
    w2_d = nc.dram_tensor("w2d", [128, 2], BF16, kind="ExternalInput")
    mag_d = nc.dram_tensor("mag", [NBANK, 128, 512], BF16, kind="ExternalOutput")

    with tile.TileContext(nc) as tc:
        with (
            tc.tile_pool(name="wp", bufs=1) as wp,
            tc.tile_pool(name="xp", bufs=3) as xp,
            tc.tile_pool(name="ep", bufs=2) as ep,
            tc.tile_pool(name="hp", bufs=3) as hp,
            tc.tile_pool(name="mp", bufs=3) as mp,
            tc.tile_pool(name="zp", bufs=2, space="PSUM") as zp,
            tc.tile_pool(name="magp", bufs=2, space="PSUM") as magp,
        ):
            w1a = wp.tile([128, 128], BF16, tag="w1a")
            w1b = wp.tile([128, 128], BF16, tag="w1b")
            b1s = wp.tile([128, 1], F32, tag="b1s")
            w2d = wp.tile([128, 2], BF16, tag="w2d")
            nc.gpsimd.dma_start(w1a[:], w1a_d[:])
            nc.gpsimd.dma_start(w1b[:], w1b_d[:])
            nc.gpsimd.dma_start(b1s[:], b1_d[:])
            nc.gpsimd.dma_start(w2d[:], w2_d[:])

            # per-group pending mm2 work: (h_tile, list of (slice, sc))
            pending = None
            mag_t = None
            mag_sb = None

            def emit_mm2(pend):
                nonlocal mag_t, mag_sb
                h_t, scs = pend
                for si, sc_abs in enumerate(scs):
                    slot = sc_abs % 4
                    if slot == 0:
                        mag_t = magp.tile([128, 512], F32, tag="magt")
                    off = 32 * slot
                    nc.tensor.matmul(
                        mag_t[off:off + 2, :], w2d[:],
                        h_t[:, si * 512:(si + 1) * 512],
                        start=True, stop=True, tile_position=(0, off),
                    )
                    if slot == 3 or sc_abs == NSC - 1:
                        bank = sc_abs // 4
                        mag_sb = mp.tile([128, 512], BF16, tag="magsb")
                        nc.vector.tensor_copy(mag_sb[:], mag_t[:])
                        nc.gpsimd.dma_start(mag_d[bank], mag_sb[:])

            sc_abs = 0
            for ti, size in enumerate(TILE_SIZES):
                width = size * SC
                xt = xp.tile([128, XT_TILE], BF16, tag="xt")
                nc.sync.dma_start(xt[:, :width], xt_d[ti, :, :width])
                for g in range(size // GRP):
                    base = g * GRP * SC
                    z = zp.tile([128, GRP * 512], F32, tag="z")
                    # mm1: A-halves with w1a stationary, then B-halves w1b
                    for s in range(GRP):
                        nc.tensor.matmul(
                            z[:, s * 512:(s + 1) * 512], w1a[:],
                            xt[:, base + s * SC:base + s * SC + 512],
                            start=True, stop=False,
                        )
                    for s in range(GRP):
                        nc.tensor.matmul(
                            z[:, s * 512:(s + 1) * 512], w1b[:],
                            xt[:, base + s * SC + 512:base + s * SC + 1024],
                            start=False, stop=True,
                        )
                    # pipelined mm2 of the previous group (h ready by now)
                    if pending is not None:
                        emit_mm2(pending)
                    # softplus: Exp then Ln, one wide op each
                    e_t = ep.tile([128, GRP * 512], F32, tag="e")
                    nc.scalar.activation(e_t[:], z[:], Exp, bias=b1s[:, :1])
                    h_t = hp.tile([128, GRP * 512], BF16, tag="h")
                    nc.scalar.activation(h_t[:], e_t[:], Ln, bias=1.0)
                    pending = (h_t, [sc_abs + s for s in range(GRP)])
                    sc_abs += GRP
            emit_mm2(pending)
    nc.compile()
    return nc


def _get_nc():
    if "nc" not in _CACHE:
        _CACHE["nc"] = _build_nc()
    return _CACHE["nc"]


def kernel(features, edge_vectors, edge_lengths, edge_index, w1, b1, w2, b2):
    global LAST_RESULTS
    from concourse.bass_utils import run_bass_kernel_spmd

    BF = ml_dtypes.bfloat16
    features = np.asarray(features, dtype=np.float32)
    edge_vectors = np.asarray(edge_vectors, dtype=np.float32)
    edge_lengths = np.asarray(edge_lengths, dtype=np.float32)
    edge_index = np.asarray(edge_index)
    w1 = np.asarray(w1, dtype=np.float32)
    b1 = np.asarray(b1, dtype=np.float32).reshape(-1)
    w2 = np.asarray(w2, dtype=np.float32).reshape(-1, 1)
    b2 = np.asarray(b2, dtype=np.float32).reshape(-1)

    # replicated small weights, padded for the stacked-z / block-diag tricks
    w1a = np.zeros((128, 128), np.float32)
    w1a[:, :H_DIM] = w1
    w1b = np.zeros((128, 128), np.float32)
    w1b[:, H_DIM:] = w1
    b1s = np.concatenate([b1, b1]).astype(np.float32).reshape(128, 1)
    w2d = np.zeros((128, 2), np.float32)
    w2d[:H_DIM, 0] = w2[:, 0]
    w2d[H_DIM:, 1] = w2[:, 0]

    feats_bf = features.astype(BF)

    # shard edges contiguously across cores; per-core transposed bf16 panel
    in_maps = []
    for c in range(N_CORES):
        sl = slice(c * EC, (c + 1) * EC)
        panel = np.zeros((128, ECP), BF)
        panel[:, :EC] = feats_bf[sl].T
        xt = np.zeros((NTILES, 128, XT_TILE), BF)
        a = 0
        for ti, size in enumerate(TILE_SIZES):
            w = size * SC
            xt[ti, :, :w] = panel[:, a:a + w]
            a += w
        in_maps.append({
            "xt": xt,
            "w1a": w1a.astype(BF), "w1b": w1b.astype(BF),
            "b1s": b1s, "w2d": w2d.astype(BF),
        })

    nc = _get_nc()
    try:
        res = run_bass_kernel_spmd(nc, in_maps, core_ids=list(range(N_CORES)))
    except Exception:
        # one retry for transient runtime failures
        import time
        time.sleep(2.0)
        res = run_bass_kernel_spmd(nc, in_maps, core_ids=list(range(N_CORES)))
    LAST_RESULTS = res

    # decode mag: [NBANK, 128, 512] bf16; sc = 4*bank + slot, partition rows
    # 32*slot (+0 = A-edges col c -> edge sc*1024+c, +1 = B-edges +512)
    mag = np.empty(E_TOT, np.float32)
    for c in range(N_CORES):
        arr = np.asarray(res.results[c]["mag"], dtype=BF).astype(np.float32)
        mr = arr.reshape(NBANK, 4, 32, 512)[:, :, :2, :].reshape(NBANK * 4, 2, 512)
        mag[c * EC:(c + 1) * EC] = mr[:NSC].reshape(-1)[:EC]

    # fold b2 and the shifted-softplus constant: h_ref = h_dev - log(2)
    mag = mag + (b2[0] - np.float32(np.log(2.0)) * w2.sum())

    center = edge_index[0].astype(np.int64)
    neigh = edge_index[1].astype(np.int64)

    # scatter-mean debias per center atom
    cnt = np.bincount(center, minlength=N_ATOMS).astype(np.float32)
    ssum = np.bincount(center, weights=mag.astype(np.float64), minlength=N_ATOMS)
    bias = (ssum / np.maximum(cnt, 1.0)).astype(np.float32)
    mag = mag - bias[center]

    # pair-averaged antisymmetric force assembly (see module docstring)
    unit = edge_vectors / edge_lengths[:, None]
    val = (0.5 * mag)[:, None] * unit  # [E, 3]
    forces = np.zeros((N_ATOMS, 3), np.float32)
    for k in range(3):
        fc = np.bincount(center, weights=val[:, k].astype(np.float64), minlength=N_ATOMS)
        fn = np.bincount(neigh, weights=val[:, k].astype(np.float64), minlength=N_ATOMS)
        forces[:, k] = (fc - fn).astype(np.float32)
    return forces
